# revision 13
# baseline (speedup 1.0000x reference)
"""Trainium2 Bass kernel for nn_AttentionHead (B=4, S=2048, E=2048, DH=256).

Sharding: 8 cores = (batch b, query-half h). Each core computes attention for
1024 queries over all 2048 keys of its batch (K/V projections duplicated
across the pair of cores sharing a batch; data-parallel otherwise).

Query ownership is INTERLEAVED at 128-row block granularity (core h owns
global blocks {h, h+2, ...}) and the host passes x[b].T with key columns
permuted own-blocks-first. That makes the causal structure near-identical on
every core, so the SPMD program statically skips fully-masked score tiles.
The residual h-asymmetry (other-core key block j is valid from query tile
j+1 on h=0 but from j on h=1) plus key-padding is folded into a tiny
per-exp-instruction bias table ([128, NI] f32): softmax masking costs no
score-sized DMA at all. The causal triangle on diagonal tiles is a single
[128,128] additive constant.

All heavy matmuls run in bf16 (full PE rate at any tile size); accumulation
stays fp32 in PSUM. Scores are computed transposed (S^T[k, q]) so the softmax
denominator comes from an all-ones matmul; softmax max-subtraction is skipped
(scores are bounded far below fp32 exp overflow).

Attention runs in 4 query groups of 256 so the final-group epilogue
(reciprocal + transpose + scale + store) is short.
"""
import sys

sys.path.insert(0, "/opt/trn_rl_repo")

import numpy as np

import concourse.bacc as bacc
import concourse.mybir as mybir
import concourse.tile as tile

F32 = mybir.dt.float32
F32R = mybir.dt.float32r
BF16 = mybir.dt.bfloat16
AF = mybir.ActivationFunctionType
ALU = mybir.AluOpType

B, S, E, DH = 4, 2048, 2048, 256
SQ = S // 2          # queries per core
EC = E // 128        # contraction chunks (16)
CG = 512             # projection column-group width
NCG = S // CG        # projection column groups (4)
GQ = 256             # attention query-group width
NG = SQ // GQ        # attention query groups (4)
NEG = -1.0e5         # additive mask (exp(-1e5/16) == 0 in f32)
SCALE = 1.0 / 16.0   # 1/sqrt(DH)

TRACE = False
LAST_RESULTS = None

_prog_cache = None


def core_key_order(h):
    """(qidx, kidx): global query rows owned by core h, and its key order."""
    own = np.concatenate(
        [np.arange((2 * j + h) * 128, (2 * j + h + 1) * 128) for j in range(8)]
    )
    other = np.concatenate(
        [np.arange((2 * j + 1 - h) * 128, (2 * j + 2 - h) * 128) for j in range(8)]
    )
    return own, np.concatenate([own, other])


def group_tiles(g):
    """Score tiles for query group g (local query tiles t0=2g, t1=2g+1).

    Returns a list of (p, qs, n, tri, segs): key-tile position p, group-column
    start qs, width n, whether the causal triangle applies (always at local
    cols 0..128 when True), and exp segments [(group_colstart, width, kind)]
    with kind 'pad' (key padding only) or 'kill' (whole block masked on h=0,
    padding-only on h=1).
    """
    t0, t1 = 2 * g, 2 * g + 1
    out = []
    for p in range(8):            # own keys: valid for t >= p, diagonal at t==p
        if p < t0:
            out.append((p, 0, 256, False, [(0, 256, "pad")]))
        elif p == t0:
            out.append((p, 0, 256, True, [(0, 256, "pad")]))
        elif p == t1:
            out.append((p, 128, 128, True, [(128, 128, "pad")]))
    for j in range(8):            # other keys: valid for t >= j+1 (h=0) / t >= j (h=1)
        p = 8 + j
        if j < t0:
            out.append((p, 0, 256, False, [(0, 256, "pad")]))
        elif j == t0:
            out.append((p, 0, 256, False, [(0, 128, "kill"), (128, 128, "pad")]))
        elif j == t1:
            out.append((p, 128, 128, False, [(128, 128, "kill")]))
    return out


def bias_columns():
    """Flat (p, kind) per bias-table column, in device emit order."""
    cols = []
    for g in range(NG):
        for p, qs, n, tri, segs in group_tiles(g):
            for cs, w, kind in segs:
                cols.append((p, kind))
    return cols


NI = len(bias_columns())


def _build_program():
    nc = bacc.Bacc("TRN2", target_bir_lowering=False, debug=False, num_devices=8)

    # host-pre-tiled: xT[p, cg, e, s] = x.T[e*128+p, cg*512+s] so every
    # projection chunk DMA is contiguous per partition (4KB descriptors)
    xT = nc.dram_tensor("xT", [128, NCG * EC * CG], BF16,
                        kind="ExternalInput").ap()
    wq = nc.dram_tensor("wq", [128, EC * DH], BF16, kind="ExternalInput").ap()
    wk = nc.dram_tensor("wk", [128, EC * DH], BF16, kind="ExternalInput").ap()
    wv = nc.dram_tensor("wv", [128, EC * DH], BF16, kind="ExternalInput").ap()
    bq = nc.dram_tensor("bq", [128, 2], F32, kind="ExternalInput").ap()
    bk = nc.dram_tensor("bk", [128, 2], F32, kind="ExternalInput").ap()
    bvb = nc.dram_tensor("bvb", [128, DH], F32, kind="ExternalInput").ap()
    onesm = nc.dram_tensor("onesm", [128, 128], BF16, kind="ExternalInput").ap()
    ident = nc.dram_tensor("ident", [128, 128], BF16, kind="ExternalInput").ap()
    trim = nc.dram_tensor("trim", [128, 128], F32, kind="ExternalInput").ap()
    btab = nc.dram_tensor("btab", [128, NI], F32, kind="ExternalInput").ap()
    e0 = nc.dram_tensor("e0", [128, 8], F32R, kind="ExternalInput").ap()
    out = nc.dram_tensor("out", [SQ, DH], F32, kind="ExternalOutput").ap()

    with tile.TileContext(nc) as tc:
        _emit(nc, tc, xT, wq, wk, wv, bq, bk, bvb, onesm, ident, trim, btab,
              e0, out)
    nc.compile()
    return nc


def _emit(nc, tc, xT, wq, wk, wv, bq, bk, bvb, onesm, ident, trim, btab, e0,
          out):
    from contextlib import ExitStack

    with ExitStack() as ctx:
        const = ctx.enter_context(tc.tile_pool(name="const", bufs=1))
        persist = ctx.enter_context(tc.tile_pool(name="persist", bufs=1))

        # PE p-state warmup: dummy matmuls on a memset tile, queued before
        # everything else so the clock is fully ramped (2.4 GHz) when the
        # first real matmul's inputs land; runs in the DMA preamble shadow.
        warm = const.tile([128, 512], BF16, tag="warm")
        nc.gpsimd.memset(warm[:], 0.0)
        wps, wps_free = tc.tile([128, 512], F32, space="PSUM", name="warmps")
        for _ in range(8):
            nc.tensor.matmul(wps[:], warm[:, :128], warm[:], start=True,
                             stop=True)
        wps_free()

        # ---- persistent SBUF tensors -----------------------------------
        wq_sb = const.tile([128, EC, DH], BF16, tag="wq")
        wk_sb = const.tile([128, EC, DH], BF16, tag="wk")
        wv_sb = const.tile([128, EC, DH], BF16, tag="wv")
        # weights go on the ACT HWDGE ring (overlapping the xT loads on the
        # SP ring); wk first, in 4 chunks, so the first matmuls start early
        wk_r = wk.rearrange("p (c d) -> p c d", c=EC)
        for ep in range(4):
            nc.scalar.dma_start(wk_sb[:, ep * 4:(ep + 1) * 4, :],
                                wk_r[:, ep * 4:(ep + 1) * 4, :])
        wq_r = wq.rearrange("p (c d) -> p c d", c=EC)
        for ep in range(2):
            nc.scalar.dma_start(wq_sb[:, ep * 8:(ep + 1) * 8, :],
                                wq_r[:, ep * 8:(ep + 1) * 8, :])
        wv_r = wv.rearrange("p (c d) -> p c d", c=EC)
        for ep in range(2):
            nc.scalar.dma_start(wv_sb[:, ep * 8:(ep + 1) * 8, :],
                                wv_r[:, ep * 8:(ep + 1) * 8, :])

        # small constants on the gpsimd SWDGE ring: zero interference
        bq_sb = const.tile([128, 2], F32, tag="bq")
        bk_sb = const.tile([128, 2], F32, tag="bk")
        bvb_sb = const.tile([128, DH], F32, tag="bvb")
        onesm_sb = const.tile([128, 128], BF16, tag="onesm")
        ident_sb = const.tile([128, 128], BF16, tag="ident")
        trim_sb = const.tile([128, 128], F32, tag="trim")
        btab_sb = const.tile([128, NI], F32, tag="btab")
        e0_sb = const.tile([128, 8], F32R, tag="e0")
        nc.gpsimd.dma_start(bk_sb[:], bk[:])
        nc.gpsimd.dma_start(bq_sb[:], bq[:])
        nc.gpsimd.dma_start(btab_sb[:], btab[:])
        nc.gpsimd.dma_start(trim_sb[:], trim[:])
        nc.gpsimd.dma_start(onesm_sb[:], onesm[:])
        nc.gpsimd.dma_start(ident_sb[:], ident[:])
        nc.gpsimd.dma_start(e0_sb[:], e0[:])
        nc.gpsimd.dma_start(bvb_sb[:], bvb[:])

        kt_sb = persist.tile([128, 2, S], BF16, tag="ktp")
        qt_sb = persist.tile([128, 2, SQ], BF16, tag="qtp")
        v_sb = persist.tile([128, S // 128, DH], BF16, tag="vp")

        # ---- phase P: projections --------------------------------------
        xT_r = xT.rearrange("p (g c s) -> p g c s", g=NCG, c=EC)
        with tc.tile_pool(name="xt", bufs=2) as xt_pool, \
             tc.tile_pool(name="proj_ps", bufs=2, space="PSUM") as proj_ps, \
             tc.tile_pool(name="v_ps", bufs=2, space="PSUM") as v_ps:
            for cg in range(NCG):
                xt = xt_pool.tile([128, EC, CG], BF16, tag="xt")
                npc = 4 if cg == 0 else 2
                w = EC // npc
                for ep in range(npc):
                    nc.sync.dma_start(
                        xt[:, ep * w:(ep + 1) * w, :],
                        xT_r[:, cg, ep * w:(ep + 1) * w, :],
                    )
                for dh2 in range(2):
                    ps = proj_ps.tile([128, CG], F32, tag="proj")
                    for e in range(EC):
                        nc.tensor.matmul(
                            ps[:],
                            wk_sb[:, e, dh2 * 128:(dh2 + 1) * 128],
                            xt[:, e, :],
                            start=(e == 0),
                            stop=(e == EC - 1),
                        )
                    nc.scalar.activation(
                        kt_sb[:, dh2, cg * CG:(cg + 1) * CG], ps[:],
                        AF.Identity, bias=bk_sb[:, dh2:dh2 + 1],
                    )
                if cg * CG < SQ:
                    for dh2 in range(2):
                        ps = proj_ps.tile([128, CG], F32, tag="proj")
                        for e in range(EC):
                            nc.tensor.matmul(
                                ps[:],
                                wq_sb[:, e, dh2 * 128:(dh2 + 1) * 128],
                                xt[:, e, :],
                                start=(e == 0),
                                stop=(e == EC - 1),
                            )
                        nc.scalar.activation(
                            qt_sb[:, dh2, cg * CG:(cg + 1) * CG], ps[:],
                            AF.Identity, bias=bq_sb[:, dh2:dh2 + 1],
                        )
                for kt4 in range(CG // 128):
                    kt = cg * (CG // 128) + kt4
                    psv = v_ps.tile([128, DH], F32, tag="vps")
                    for e in range(EC):
                        nc.tensor.matmul(
                            psv[:],
                            xt[:, e, kt4 * 128:(kt4 + 1) * 128],
                            wv_sb[:, e, :],
                            start=(e == 0),
                            stop=(e == EC - 1),
                        )
                    nc.vector.tensor_copy(v_sb[:, kt, :], psv[:])

        # ---- phase A: attention + per-group finalize -------------------
        bi = 0  # running bias-table column index (must match bias_columns())
        # PSUM budget: slots round up to whole banks, 8 total:
        # s_ps 3 + o_ps 2 (two tags) + d_ps 1 + f_ps 1 + r_ps 1 = 8
        with tc.tile_pool(name="s_ps", bufs=3, space="PSUM") as s_ps, \
             tc.tile_pool(name="o_ps", bufs=1, space="PSUM") as o_ps, \
             tc.tile_pool(name="d_ps", bufs=1, space="PSUM") as d_ps, \
             tc.tile_pool(name="f_ps", bufs=1, space="PSUM") as f_ps, \
             tc.tile_pool(name="r_ps", bufs=1, space="PSUM") as r_ps, \
             tc.tile_pool(name="pt", bufs=3) as pt_pool, \
             tc.tile_pool(name="fin", bufs=2) as fin:
            for g in range(NG):
                q0 = g * GQ
                tiles = group_tiles(g)
                otp = [
                    o_ps.tile([128, GQ], F32, tag=f"ot{d}", name=f"otp{g}_{d}")
                    for d in range(2)
                ]
                dnp = d_ps.tile([128, GQ], F32, tag="dn")
                last = len(tiles) - 1
                for ti, (p, qs, n, tri, segs) in enumerate(tiles):
                    sp = s_ps.tile([128, GQ], F32, tag="sp")
                    for dh2 in range(2):
                        nc.tensor.matmul(
                            sp[:, :n],
                            kt_sb[:, dh2, p * 128:(p + 1) * 128],
                            qt_sb[:, dh2, q0 + qs:q0 + qs + n],
                            start=(dh2 == 0),
                            stop=(dh2 == 1),
                        )
                    if tri:
                        nc.vector.tensor_tensor(
                            sp[:, :128], sp[:, :128], trim_sb[:], op=ALU.add
                        )
                    pt = pt_pool.tile([128, GQ], BF16, tag="pt")
                    for cs, w, kind in segs:
                        nc.scalar.activation(
                            pt[:, cs - qs:cs - qs + w],
                            sp[:, cs - qs:cs - qs + w],
                            AF.Exp, scale=SCALE, bias=btab_sb[:, bi:bi + 1],
                        )
                        bi += 1
                    for dh2 in range(2):
                        nc.tensor.matmul(
                            otp[dh2][:, qs:qs + n],
                            v_sb[:, p, dh2 * 128:(dh2 + 1) * 128],
                            pt[:, :n],
                            start=(ti == 0),
                            stop=(ti == last),
                        )
                    nc.tensor.matmul(
                        dnp[:, qs:qs + n],
                        onesm_sb[:],
                        pt[:, :n],
                        start=(ti == 0),
                        stop=(ti == last),
                    )

                # finalize this query group: recip + transpose + scale + store
                ot_g = fin.tile([128, 2, GQ], BF16, tag="otg")
                for dh2 in range(2):
                    nc.vector.tensor_copy(ot_g[:, dh2, :], otp[dh2][:])
                dn_g = fin.tile([128, GQ], F32R, tag="dng")
                nc.vector.tensor_copy(dn_g[:], dnp[:])
                rtp = r_ps.tile([128, 16], F32, tag="rt", name=f"rtp{g}")
                for qi in range(2):
                    nc.tensor.matmul(
                        rtp[:, qi * 8:(qi + 1) * 8],
                        dn_g[:, qi * 128:(qi + 1) * 128],
                        e0_sb[:],
                        start=(qi == 0),
                        stop=(qi == 1),
                    )
                rt_sb = fin.tile([128, 16], F32, tag="rtsb")
                nc.vector.tensor_copy(rt_sb[:], rtp[:])
                rc_sb = fin.tile([128, 16], F32, tag="rcsb")
                nc.vector.reciprocal(rc_sb[:], rt_sb[:])
                for qi in range(2):
                    qt = g * 2 + qi
                    ofp = f_ps.tile([128, DH], BF16, tag="of")
                    for dh2 in range(2):
                        nc.tensor.transpose(
                            ofp[:, dh2 * 128:(dh2 + 1) * 128],
                            ot_g[:, dh2, qi * 128:(qi + 1) * 128],
                            ident_sb[:],
                        )
                    ob = fin.tile([128, DH], F32, tag="ob")
                    # (ofp * 1/denom) + b_V fused on DVE, keeping the Scalar
                    # engine free for the next group's exps
                    nc.vector.scalar_tensor_tensor(
                        ob[:], ofp[:], rc_sb[:, qi * 8:qi * 8 + 1], bvb_sb[:],
                        op0=ALU.mult, op1=ALU.add,
                    )
                    nc.sync.dma_start(out[qt * 128:(qt + 1) * 128, :], ob[:])


def _get_program():
    global _prog_cache
    if _prog_cache is None:
        _prog_cache = _build_program()
    return _prog_cache


def kernel(x, causal_mask, padding_mask, W_Q, b_Q, W_K, b_K, W_V, b_V):
    global LAST_RESULTS
    from concourse.bass_utils import run_bass_kernel_spmd

    import ml_dtypes

    bf16 = ml_dtypes.bfloat16
    x = np.asarray(x, dtype=np.float32)
    pad = np.asarray(padding_mask)                   # [B, S]  True = masked key

    def tile_w(W):
        W = np.asarray(W, dtype=np.float32)
        return np.ascontiguousarray(
            W.reshape(EC, 128, DH).transpose(1, 0, 2).reshape(128, EC * DH)
        ).astype(bf16)

    W_Qb = tile_w(W_Q)
    W_Kb = tile_w(W_K)
    W_Vb = tile_w(W_V)
    bvb = np.ascontiguousarray(
        np.broadcast_to(np.asarray(b_V, dtype=np.float32), (128, DH))
    )
    bqh = np.ascontiguousarray(np.asarray(b_Q, dtype=np.float32).reshape(2, 128).T)
    bkh = np.ascontiguousarray(np.asarray(b_K, dtype=np.float32).reshape(2, 128).T)
    e0v = np.zeros((128, 8), dtype=np.float32)
    e0v[0, :] = 1.0
    onesm = np.ones((128, 128), dtype=bf16)
    identm = np.eye(128, dtype=np.float32).astype(bf16)
    q = np.arange(128)
    trimv = np.where(q[None, :] >= q[:, None], np.float32(0.0),
                     np.float32(NEG))  # [k, q]: attend iff q >= k
    trimv = np.ascontiguousarray(trimv)

    cols = bias_columns()
    in_maps = []
    for c in range(8):
        b, h = c // 2, c % 2
        qidx, kidx = core_key_order(h)
        xT = x[b][kidx].T.astype(bf16)                         # [E, S] permuted
        # pre-tile: [128, NCG, EC, CG], xTt[p, g, e, s] = xT[e*128+p, g*512+s]
        xTt = np.ascontiguousarray(
            xT.reshape(EC, 128, NCG, CG).transpose(1, 2, 0, 3)
        ).reshape(128, NCG * EC * CG)
        padp = pad[b][kidx].reshape(16, 128)                   # [tile, row]
        bt = np.zeros((128, len(cols)), dtype=np.float32)
        for i, (p, kind) in enumerate(cols):
            col = np.where(padp[p], np.float32(NEG), np.float32(0.0))
            if kind == "kill" and h == 0:
                col = np.full(128, np.float32(NEG))
            bt[:, i] = col
        in_maps.append({
            "xT": xTt,
            "wq": W_Qb, "wk": W_Kb, "wv": W_Vb,
            "bq": bqh, "bk": bkh, "bvb": bvb,
            "onesm": onesm, "ident": identm, "trim": trimv,
            "btab": np.ascontiguousarray(bt), "e0": e0v,
        })

    nc = _get_program()
    res = run_bass_kernel_spmd(nc, in_maps, list(range(8)), trace=TRACE)
    LAST_RESULTS = res

    outp = np.empty((B, S, DH), dtype=np.float32)
    for c in range(8):
        b, h = c // 2, c % 2
        qidx, _ = core_key_order(h)
        outp[b][qidx] = res.results[c]["out"]
    return outp


# revision 14
# speedup vs baseline: 1.5527x; 1.5527x over previous
"""Trainium2 Bass kernel for nn_AttentionHead (B=4, S=2048, E=2048, DH=256).

Sharding: 8 cores = (batch b, query-half h). Core h owns the interleaved
query blocks {h, h+2, ...} of its batch (1024 queries over all 2048 keys).

K/V projections are computed ONCE per pair: each core projects K/V only for
its own 1024 key columns, then the halves are exchanged with a pairwise HBM
AllGather (replica groups {2b, 2b+1}), pipelined in two rounds so the gather
overlaps the Q projection and the second half of K/V compute. Keys are stored
RANK-ORDERED (even global blocks at positions 0..7, odd at 8..15, identical
on both cores) so the gather output maps SPMD-symmetrically into SBUF.

Causal structure per (query tile t, key tile): even tile p is included for
p <= t with a boundary mask M1 at p == t (h=0: triangle, h=1: all-valid);
odd tile j is included for j <= t with M2 at j == t (h=0: fully masked,
h=1: triangle). M1/M2 are tiny per-core input constants; key padding is a
per-key-tile bias column applied inside the softmax Exp activation. No
score-sized mask traffic at all.

All heavy matmuls run in bf16 (full PE rate at any tile size); accumulation
stays fp32 in PSUM. Scores are computed transposed (S^T[k, q]) so the softmax
denominator comes from an all-ones matmul; softmax max-subtraction is skipped
(scores are bounded far below fp32 exp overflow).

Attention runs in 4 query groups of 256 so the final-group epilogue is short.
"""
import sys

sys.path.insert(0, "/opt/trn_rl_repo")

import numpy as np

import concourse.bacc as bacc
import concourse.mybir as mybir
import concourse.tile as tile

F32 = mybir.dt.float32
F32R = mybir.dt.float32r
BF16 = mybir.dt.bfloat16
AF = mybir.ActivationFunctionType
ALU = mybir.AluOpType

B, S, E, DH = 4, 2048, 2048, 256
SQ = S // 2          # queries per core
EC = E // 128        # contraction chunks (16)
CG = 512             # projection column-group width
NCG = SQ // CG       # projection column groups over own keys (2)
GQ = 256             # attention query-group width
NG = SQ // GQ        # attention query groups (4)
NEG = -1.0e5         # additive mask (exp(-1e5/16) == 0 in f32)
SCALE = 1.0 / 16.0   # 1/sqrt(DH)
PAIRS = [[0, 1], [2, 3], [4, 5], [6, 7]]

TRACE = False
LAST_RESULTS = None

_prog_cache = None


def core_key_order(h):
    """(qidx, kidx): global query rows owned by core h, and the rank-ordered
    key rows (identical for both cores of a pair)."""
    own = np.concatenate(
        [np.arange((2 * j + h) * 128, (2 * j + h + 1) * 128) for j in range(8)]
    )
    even = np.concatenate(
        [np.arange(2 * j * 128, (2 * j + 1) * 128) for j in range(8)]
    )
    odd = np.concatenate(
        [np.arange((2 * j + 1) * 128, (2 * j + 2) * 128) for j in range(8)]
    )
    return own, np.concatenate([even, odd])


def group_tiles(g):
    """Score tiles for query group g (local query tiles t0=2g, t1=2g+1).

    Returns (p, qs, n, m): key-tile position p, group-column start qs, width
    n, and boundary mask m in {None, 'M1', 'M2'} applied at local cols 0:128.
    """
    t0, t1 = 2 * g, 2 * g + 1
    out = []
    for p in range(8):            # even key blocks: include p <= t
        if p < t0:
            out.append((p, 0, 256, None))
        elif p == t0:
            out.append((p, 0, 256, "M1"))
        elif p == t1:
            out.append((p, 128, 128, "M1"))
    for j in range(8):            # odd key blocks: include j <= t
        p = 8 + j
        if j < t0:
            out.append((p, 0, 256, None))
        elif j == t0:
            out.append((p, 0, 256, "M2"))
        elif j == t1:
            out.append((p, 128, 128, "M2"))
    return out


def _build_program():
    nc = bacc.Bacc("TRN2", target_bir_lowering=False, debug=False, num_devices=8)

    # host-pre-tiled: xT[p, cg, e, s] = x_own.T[e*128+p, cg*512+s] so every
    # projection chunk DMA is contiguous per partition (4KB descriptors)
    xT = nc.dram_tensor("xT", [128, NCG * EC * CG], BF16,
                        kind="ExternalInput").ap()
    wq = nc.dram_tensor("wq", [128, EC * DH], BF16, kind="ExternalInput").ap()
    wk = nc.dram_tensor("wk", [128, EC * DH], BF16, kind="ExternalInput").ap()
    wv = nc.dram_tensor("wv", [128, EC * DH], BF16, kind="ExternalInput").ap()
    bq = nc.dram_tensor("bq", [128, 2], F32, kind="ExternalInput").ap()
    bk = nc.dram_tensor("bk", [128, 2], F32, kind="ExternalInput").ap()
    bvb = nc.dram_tensor("bvb", [128, DH], F32, kind="ExternalInput").ap()
    onesm = nc.dram_tensor("onesm", [128, 128], BF16, kind="ExternalInput").ap()
    ident = nc.dram_tensor("ident", [128, 128], BF16, kind="ExternalInput").ap()
    m1 = nc.dram_tensor("m1", [128, 128], F32, kind="ExternalInput").ap()
    m2 = nc.dram_tensor("m2", [128, 128], F32, kind="ExternalInput").ap()
    padt = nc.dram_tensor("padt", [128, 16], F32, kind="ExternalInput").ap()
    e0 = nc.dram_tensor("e0", [128, 8], F32R, kind="ExternalInput").ap()
    # K/V exchange staging: per round r, 2048 bf16 per partition
    # ([dh2=0 512 | dh2=1 512 | v 4x256]); gth{r} is rank-major gather output
    kvs = [nc.dram_tensor(f"kvs{r}", [128, 2048], BF16, kind="Internal").ap()
           for r in range(2)]
    gth = [nc.dram_tensor(f"gth{r}", [2, 128, 2048], BF16,
                          kind="Internal").ap() for r in range(2)]
    out = nc.dram_tensor("out", [SQ, DH], F32, kind="ExternalOutput").ap()

    with tile.TileContext(nc) as tc:
        _emit(nc, tc, xT, wq, wk, wv, bq, bk, bvb, onesm, ident, m1, m2, padt,
              e0, kvs, gth, out)
    nc.compile()
    return nc


def _emit(nc, tc, xT, wq, wk, wv, bq, bk, bvb, onesm, ident, m1, m2, padt,
          e0, kvs, gth, out):
    from contextlib import ExitStack

    with ExitStack() as ctx:
        const = ctx.enter_context(tc.tile_pool(name="const", bufs=1))
        persist = ctx.enter_context(tc.tile_pool(name="persist", bufs=1))

        # PE p-state warmup: dummy matmuls on a memset tile, queued before
        # everything else so the clock is fully ramped (2.4 GHz) when the
        # first real matmul's inputs land; runs in the DMA preamble shadow.
        warm = const.tile([128, 512], BF16, tag="warm")
        nc.gpsimd.memset(warm[:], 0.0)
        wps, wps_free = tc.tile([128, 512], F32, space="PSUM", name="warmps")
        for _ in range(8):
            nc.tensor.matmul(wps[:], warm[:, :128], warm[:], start=True,
                             stop=True)
        wps_free()

        # ---- persistent SBUF tensors -----------------------------------
        wq_sb = const.tile([128, EC, DH], BF16, tag="wq")
        wk_sb = const.tile([128, EC, DH], BF16, tag="wk")
        wv_sb = const.tile([128, EC, DH], BF16, tag="wv")
        # weights go on the ACT HWDGE ring (overlapping the xT loads on the
        # SP ring); wk first, in 4 chunks, so the first matmuls start early
        wk_r = wk.rearrange("p (c d) -> p c d", c=EC)
        for ep in range(4):
            nc.scalar.dma_start(wk_sb[:, ep * 4:(ep + 1) * 4, :],
                                wk_r[:, ep * 4:(ep + 1) * 4, :])
        wv_r = wv.rearrange("p (c d) -> p c d", c=EC)
        for ep in range(2):
            nc.scalar.dma_start(wv_sb[:, ep * 8:(ep + 1) * 8, :],
                                wv_r[:, ep * 8:(ep + 1) * 8, :])
        wq_r = wq.rearrange("p (c d) -> p c d", c=EC)
        for ep in range(2):
            nc.scalar.dma_start(wq_sb[:, ep * 8:(ep + 1) * 8, :],
                                wq_r[:, ep * 8:(ep + 1) * 8, :])

        # small constants on the gpsimd SWDGE ring: zero interference
        bq_sb = const.tile([128, 2], F32, tag="bq")
        bk_sb = const.tile([128, 2], F32, tag="bk")
        bvb_sb = const.tile([128, DH], F32, tag="bvb")
        onesm_sb = const.tile([128, 128], BF16, tag="onesm")
        ident_sb = const.tile([128, 128], BF16, tag="ident")
        m1_sb = const.tile([128, 128], F32, tag="m1")
        m2_sb = const.tile([128, 128], F32, tag="m2")
        pad_sb = const.tile([128, 16], F32, tag="padt")
        e0_sb = const.tile([128, 8], F32R, tag="e0")
        nc.gpsimd.dma_start(bk_sb[:], bk[:])
        nc.gpsimd.dma_start(bq_sb[:], bq[:])
        nc.gpsimd.dma_start(pad_sb[:], padt[:])
        nc.gpsimd.dma_start(m1_sb[:], m1[:])
        nc.gpsimd.dma_start(m2_sb[:], m2[:])
        nc.gpsimd.dma_start(onesm_sb[:], onesm[:])
        nc.gpsimd.dma_start(ident_sb[:], ident[:])
        nc.gpsimd.dma_start(e0_sb[:], e0[:])
        nc.gpsimd.dma_start(bvb_sb[:], bvb[:])

        kt_own = persist.tile([128, 2, SQ], BF16, tag="kto")
        v_own = persist.tile([128, 8, DH], BF16, tag="vo")
        qt_sb = persist.tile([128, 2, SQ], BF16, tag="qtp")
        kt_sb = persist.tile([128, 2, S], BF16, tag="ktp")
        v_sb = persist.tile([128, S // 128, DH], BF16, tag="vp")

        # ---- phase P: projections + pipelined K/V exchange -------------
        xT_r = xT.rearrange("p (g c s) -> p g c s", g=NCG, c=EC)
        xts = []
        with tc.tile_pool(name="xt", bufs=2) as xt_pool, \
             tc.tile_pool(name="proj_ps", bufs=2, space="PSUM") as proj_ps, \
             tc.tile_pool(name="v_ps", bufs=2, space="PSUM") as v_ps:
            for cg in range(NCG):
                xt = xt_pool.tile([128, EC, CG], BF16, tag="xt")
                xts.append(xt)
                npc = 4 if cg == 0 else 2
                w = EC // npc
                for ep in range(npc):
                    nc.sync.dma_start(
                        xt[:, ep * w:(ep + 1) * w, :],
                        xT_r[:, cg, ep * w:(ep + 1) * w, :],
                    )
                for dh2 in range(2):
                    ps = proj_ps.tile([128, CG], F32, tag="proj")
                    for e in range(EC):
                        nc.tensor.matmul(
                            ps[:],
                            wk_sb[:, e, dh2 * 128:(dh2 + 1) * 128],
                            xt[:, e, :],
                            start=(e == 0),
                            stop=(e == EC - 1),
                        )
                    nc.scalar.activation(
                        kt_own[:, dh2, cg * CG:(cg + 1) * CG], ps[:],
                        AF.Identity, bias=bk_sb[:, dh2:dh2 + 1],
                    )
                for kt4 in range(CG // 128):
                    kt = cg * (CG // 128) + kt4
                    psv = v_ps.tile([128, DH], F32, tag="vps")
                    for e in range(EC):
                        nc.tensor.matmul(
                            psv[:],
                            xt[:, e, kt4 * 128:(kt4 + 1) * 128],
                            wv_sb[:, e, :],
                            start=(e == 0),
                            stop=(e == EC - 1),
                        )
                    nc.vector.tensor_copy(v_own[:, kt, :], psv[:])

                # stage this round's K/V half and gather it across the pair
                r = cg
                for dh2 in range(2):
                    nc.sync.dma_start(
                        kvs[r][:, dh2 * 512:(dh2 + 1) * 512],
                        kt_own[:, dh2, r * 512:(r + 1) * 512],
                    )
                nc.sync.dma_start(
                    kvs[r][:, 1024:2048], v_own[:, 4 * r:4 * r + 4, :]
                )
                nc.gpsimd.collective_compute(
                    "AllGather",
                    ALU.bypass,
                    replica_groups=PAIRS,
                    ins=[kvs[r][:]],
                    outs=[gth[r][:]],
                )
                # readback: rank rk's half covers key positions 8*rk+4r..+3
                for rk in range(2):
                    c0 = (8 * rk + 4 * r) * 128
                    for dh2 in range(2):
                        nc.sync.dma_start(
                            kt_sb[:, dh2, c0:c0 + 512],
                            gth[r][rk, :, dh2 * 512:(dh2 + 1) * 512],
                        )
                    nc.sync.dma_start(
                        v_sb[:, 8 * rk + 4 * r:8 * rk + 4 * r + 4, :],
                        gth[r][rk, :, 1024:2048],
                    )

            # Q projection (overlaps the gathers)
            for cg in range(NCG):
                for dh2 in range(2):
                    ps = proj_ps.tile([128, CG], F32, tag="proj")
                    for e in range(EC):
                        nc.tensor.matmul(
                            ps[:],
                            wq_sb[:, e, dh2 * 128:(dh2 + 1) * 128],
                            xts[cg][:, e, :],
                            start=(e == 0),
                            stop=(e == EC - 1),
                        )
                    nc.scalar.activation(
                        qt_sb[:, dh2, cg * CG:(cg + 1) * CG], ps[:],
                        AF.Identity, bias=bq_sb[:, dh2:dh2 + 1],
                    )

        # ---- phase A: attention + per-group finalize -------------------
        # PSUM budget: slots round up to whole banks, 8 total:
        # s_ps 3 + o_ps 2 (two tags) + d_ps 1 + f_ps 1 + r_ps 1 = 8
        with tc.tile_pool(name="s_ps", bufs=3, space="PSUM") as s_ps, \
             tc.tile_pool(name="o_ps", bufs=1, space="PSUM") as o_ps, \
             tc.tile_pool(name="d_ps", bufs=1, space="PSUM") as d_ps, \
             tc.tile_pool(name="f_ps", bufs=1, space="PSUM") as f_ps, \
             tc.tile_pool(name="r_ps", bufs=1, space="PSUM") as r_ps, \
             tc.tile_pool(name="pt", bufs=3) as pt_pool, \
             tc.tile_pool(name="fin", bufs=2) as fin:
            for g in range(NG):
                q0 = g * GQ
                tiles = group_tiles(g)
                otp = [
                    o_ps.tile([128, GQ], F32, tag=f"ot{d}", name=f"otp{g}_{d}")
                    for d in range(2)
                ]
                dnp = d_ps.tile([128, GQ], F32, tag="dn")
                last = len(tiles) - 1
                for ti, (p, qs, n, m) in enumerate(tiles):
                    sp = s_ps.tile([128, GQ], F32, tag="sp")
                    for dh2 in range(2):
                        nc.tensor.matmul(
                            sp[:, :n],
                            kt_sb[:, dh2, p * 128:(p + 1) * 128],
                            qt_sb[:, dh2, q0 + qs:q0 + qs + n],
                            start=(dh2 == 0),
                            stop=(dh2 == 1),
                        )
                    if m is not None:
                        msk = m1_sb if m == "M1" else m2_sb
                        nc.vector.tensor_tensor(
                            sp[:, :128], sp[:, :128], msk[:], op=ALU.add
                        )
                    pt = pt_pool.tile([128, GQ], BF16, tag="pt")
                    nc.scalar.activation(
                        pt[:, :n], sp[:, :n],
                        AF.Exp, scale=SCALE, bias=pad_sb[:, p:p + 1],
                    )
                    for dh2 in range(2):
                        nc.tensor.matmul(
                            otp[dh2][:, qs:qs + n],
                            v_sb[:, p, dh2 * 128:(dh2 + 1) * 128],
                            pt[:, :n],
                            start=(ti == 0),
                            stop=(ti == last),
                        )
                    nc.tensor.matmul(
                        dnp[:, qs:qs + n],
                        onesm_sb[:],
                        pt[:, :n],
                        start=(ti == 0),
                        stop=(ti == last),
                    )

                # finalize this query group: recip + transpose + scale + store
                ot_g = fin.tile([128, 2, GQ], BF16, tag="otg")
                for dh2 in range(2):
                    nc.vector.tensor_copy(ot_g[:, dh2, :], otp[dh2][:])
                dn_g = fin.tile([128, GQ], F32R, tag="dng")
                nc.vector.tensor_copy(dn_g[:], dnp[:])
                rtp = r_ps.tile([128, 16], F32, tag="rt", name=f"rtp{g}")
                for qi in range(2):
                    nc.tensor.matmul(
                        rtp[:, qi * 8:(qi + 1) * 8],
                        dn_g[:, qi * 128:(qi + 1) * 128],
                        e0_sb[:],
                        start=(qi == 0),
                        stop=(qi == 1),
                    )
                rt_sb = fin.tile([128, 16], F32, tag="rtsb")
                nc.vector.tensor_copy(rt_sb[:], rtp[:])
                rc_sb = fin.tile([128, 16], F32, tag="rcsb")
                nc.vector.reciprocal(rc_sb[:], rt_sb[:])
                for qi in range(2):
                    qt = g * 2 + qi
                    ofp = f_ps.tile([128, DH], BF16, tag="of")
                    for dh2 in range(2):
                        nc.tensor.transpose(
                            ofp[:, dh2 * 128:(dh2 + 1) * 128],
                            ot_g[:, dh2, qi * 128:(qi + 1) * 128],
                            ident_sb[:],
                        )
                    ob = fin.tile([128, DH], F32, tag="ob")
                    # (ofp * 1/denom) + b_V fused on DVE, keeping the Scalar
                    # engine free for the next group's exps
                    nc.vector.scalar_tensor_tensor(
                        ob[:], ofp[:], rc_sb[:, qi * 8:qi * 8 + 1], bvb_sb[:],
                        op0=ALU.mult, op1=ALU.add,
                    )
                    nc.scalar.dma_start(out[qt * 128:(qt + 1) * 128, :], ob[:])


def _get_program():
    global _prog_cache
    if _prog_cache is None:
        _prog_cache = _build_program()
    return _prog_cache


def kernel(x, causal_mask, padding_mask, W_Q, b_Q, W_K, b_K, W_V, b_V):
    global LAST_RESULTS
    from concourse.bass_utils import run_bass_kernel_spmd

    import ml_dtypes

    bf16 = ml_dtypes.bfloat16
    x = np.asarray(x, dtype=np.float32)
    pad = np.asarray(padding_mask)                   # [B, S]  True = masked key

    def tile_w(W):
        W = np.asarray(W, dtype=np.float32)
        return np.ascontiguousarray(
            W.reshape(EC, 128, DH).transpose(1, 0, 2).reshape(128, EC * DH)
        ).astype(bf16)

    W_Qb = tile_w(W_Q)
    W_Kb = tile_w(W_K)
    W_Vb = tile_w(W_V)
    bvb = np.ascontiguousarray(
        np.broadcast_to(np.asarray(b_V, dtype=np.float32), (128, DH))
    )
    bqh = np.ascontiguousarray(np.asarray(b_Q, dtype=np.float32).reshape(2, 128).T)
    bkh = np.ascontiguousarray(np.asarray(b_K, dtype=np.float32).reshape(2, 128).T)
    e0v = np.zeros((128, 8), dtype=np.float32)
    e0v[0, :] = 1.0
    onesm = np.ones((128, 128), dtype=bf16)
    identm = np.eye(128, dtype=np.float32).astype(bf16)
    q = np.arange(128)
    trimv = np.where(q[None, :] >= q[:, None], np.float32(0.0),
                     np.float32(NEG))  # [k, q]: attend iff q >= k
    trimv = np.ascontiguousarray(trimv)
    zerom = np.zeros((128, 128), dtype=np.float32)
    negm = np.full((128, 128), np.float32(NEG), dtype=np.float32)

    in_maps = []
    for c in range(8):
        b, h = c // 2, c % 2
        qidx, kidx = core_key_order(h)
        xT = x[b][qidx].T.astype(bf16)          # own query/key columns [E, SQ]
        # pre-tile: [128, NCG, EC, CG], xTt[p, g, e, s] = xT[e*128+p, g*512+s]
        xTt = np.ascontiguousarray(
            xT.reshape(EC, 128, NCG, CG).transpose(1, 2, 0, 3)
        ).reshape(128, NCG * EC * CG)
        padp = pad[b][kidx].reshape(16, 128)    # [rank-ordered tile, row]
        padt = np.ascontiguousarray(
            np.where(padp.T, np.float32(NEG), np.float32(0.0))
        )                                        # [128, 16]
        in_maps.append({
            "xT": xTt,
            "wq": W_Qb, "wk": W_Kb, "wv": W_Vb,
            "bq": bqh, "bk": bkh, "bvb": bvb,
            "onesm": onesm, "ident": identm,
            "m1": trimv if h == 0 else zerom,
            "m2": negm if h == 0 else trimv,
            "padt": padt, "e0": e0v,
        })

    nc = _get_program()
    res = run_bass_kernel_spmd(nc, in_maps, list(range(8)), trace=TRACE)
    LAST_RESULTS = res

    outp = np.empty((B, S, DH), dtype=np.float32)
    for c in range(8):
        b, h = c // 2, c % 2
        qidx, _ = core_key_order(h)
        outp[b][qidx] = res.results[c]["out"]
    return outp
